# revision 6
# baseline (speedup 1.0000x reference)
"""LocallyConnected1d Trainium2 kernel (8 NeuronCores, sequence-parallel).

Problem: out[b,o,l] = sum_{i,k} xpad[b,i,l+k] * w[i,o,k,l] + bias[o,l]
  B=64, Ci=Co=64, S=L=512, K=9, pad=4.

Strategy:
  * Shard out_seq_len L=512 across 8 cores (64 positions each) so the 75MB
    per-position weight tensor is moved from HBM exactly once (weight DMA is
    the roofline: ~4.7MB/core in bf16).
  * Per core, process positions in pairs (l, l+1). Contract dim is laid out
    as r = dj*64 + i (dj in {0,1}), split into 5 chunks c, where chunk c
    covers window offsets j = 2c+dj of the padded input.
  * matmul: stationary lhsT = X block [128=(dj,i), 64=b] (cheap 64-col
    LDWEIGHTS), moving rhs = weight block [128=(dj,i), 128=(l2,o)], PSUM
    out [64=b, 128=(l2,o)] accumulates over the 5 chunks.
    Weight entry at (dj,i),(l2,o) of chunk c is w[i,o,2c+dj-l2, l+l2]
    (zero if k=2c+dj-l2 outside [0,9)) -- so one pair of output positions
    is computed per PSUM tile with zero wasted streaming columns.
  * bias is folded in as a 6th rank-1 matmul: ones[1,64].T @ bias_row[1,128].
  * All operands bf16 (halves DMA + enables fast PE streaming), PSUM fp32.
"""

import sys

sys.path.insert(0, "/opt/trn_rl_repo")

import numpy as np
from ml_dtypes import bfloat16

import concourse.bass as bass
import concourse.bacc as bacc
import concourse.mybir as mybir
from concourse import tile
from concourse.bass_utils import run_bass_kernel_spmd

B = 64
CI = 64
CO = 64
S = 512
KS = 9
PAD = 4
L = 512
NCORES = 8
LS = L // NCORES          # 64 output positions per core
NPAIR = LS // 2           # 32 position pairs per core
NCH = 5                   # contract chunks per pair (j window of 10 -> 5x128)
NT = LS // 2 + NCH - 1    # 36 x-blocks of [128, 64]
OUT_GROUPS = 4            # output DMA granularity (8 pairs each)
WGRP = 4                  # pairs per weight DMA (8 DMAs of ~655KB)
NWG = NPAIR // WGRP

TRACE = False
TRACE_KW: dict = {}
LAST_RESULT = None

_cached_nc = None


def _build_nc():
    global _cached_nc
    if _cached_nc is not None:
        return _cached_nc

    nc = bacc.Bacc("TRN2", target_bir_lowering=False, debug=False,
                   num_devices=NCORES)
    bf = mybir.dt.bfloat16
    f32 = mybir.dt.float32

    xs_d = nc.dram_tensor("xs", [128, NT * 64], bf, kind="ExternalInput").ap()
    ws_d = nc.dram_tensor("ws", [NWG, 128, WGRP * NCH * 128], bf,
                          kind="ExternalInput").ap()
    bs_d = nc.dram_tensor("bs", [1, NPAIR * 128], bf, kind="ExternalInput").ap()
    out_d = nc.dram_tensor("out", [64, NPAIR * 128], f32,
                           kind="ExternalOutput").ap()

    with tile.TileContext(nc) as tc:
        with (
            tc.tile_pool(name="xp", bufs=1) as xp,
            tc.tile_pool(name="wp", bufs=NWG) as wp,
            tc.tile_pool(name="pp", bufs=8, space="PSUM") as pp,
            tc.tile_pool(name="op", bufs=OUT_GROUPS) as op,
        ):
            xs_t = xp.tile([128, NT * 64], bf, tag="xs")
            nc.sync.dma_start(xs_t[:], xs_d[:])
            bs_t = xp.tile([1, NPAIR * 128], bf, tag="bs")
            nc.scalar.dma_start(bs_t[:], bs_d[:])
            ones_t = xp.tile([1, 64], bf, tag="ones")
            nc.gpsimd.memset(ones_t[:], 1.0)

            # 8 big weight DMAs (~655KB each), alternating HWDGE rings so
            # descriptor generation isn't serialized on one sequencer.
            w_tiles = []
            for g in range(NWG):
                wt = wp.tile([128, WGRP * NCH * 128], bf, tag="wt")
                eng = nc.sync if g % 2 == 0 else nc.scalar
                eng.dma_start(wt[:], ws_d[g])
                w_tiles.append(wt)

            def w_slice(p, c):
                wt = w_tiles[p // WGRP]
                off = ((p % WGRP) * NCH + c) * 128
                return wt[:, off:off + 128]

            ppg = NPAIR // OUT_GROUPS
            out_tiles = [op.tile([64, ppg * 128], f32, tag="ot",
                                 name=f"ot{g}")
                         for g in range(OUT_GROUPS)]
            psums = [None] * NPAIR

            def finish_pair(p):
                ps = psums[p]
                nc.tensor.matmul(
                    ps[:], ones_t[:], bs_t[:, p * 128:(p + 1) * 128],
                    start=False, stop=True,
                )
                ot = out_tiles[p // ppg]
                pp_i = p % ppg
                nc.vector.tensor_copy(ot[:, pp_i * 128:(pp_i + 1) * 128],
                                      ps[:])

            # t-major: all matmuls sharing one stationary x-block are emitted
            # back-to-back (same lhsT -> redundant LDWEIGHTS are cheap/skippable,
            # PE work stays dense for HAM warm-up).
            for t in range(NT):
                for c in range(NCH):
                    p = t - c
                    if 0 <= p < NPAIR:
                        psum_t = psums[p]
                        if psum_t is None:
                            psum_t = psums[p] = pp.tile([64, 128], f32,
                                                        tag="ps",
                                                        name=f"ps{p}")
                        nc.tensor.matmul(
                            psum_t[:],
                            xs_t[:, t * 64:(t + 1) * 64],
                            w_slice(p, c),
                            start=(c == 0),
                            stop=False,
                        )
                if t >= NCH - 1:
                    finish_pair(t - (NCH - 1))

            for g in range(OUT_GROUPS):
                nc.scalar.dma_start(
                    out_d[:, g * ppg * 128:(g + 1) * ppg * 128],
                    out_tiles[g][:])

    nc.compile()
    _cached_nc = nc
    return nc


def _prep_core_inputs(xpad, weight, bias, cr):
    l0 = LS * cr
    # xs[dj*64+i, t*64+b] = xpad[b, i, l0+2t+dj]
    xsl = xpad[:, :, l0:l0 + 2 * NT]                       # [b, i, 72]
    xs = np.ascontiguousarray(
        xsl.reshape(B, CI, NT, 2).transpose(3, 1, 2, 0)    # [dj, i, t, b]
    ).reshape(128, NT * 64)

    # ws[p, dj*64+i, c*128 + l2*64 + o] = w[i,o,2c+dj-l2, l0+2p+l2]
    wsarr = np.zeros((NPAIR, 2, CI, NCH, 2, CO), np.float32)
    for c in range(NCH):
        for dj in range(2):
            for l2 in range(2):
                k = 2 * c + dj - l2
                if 0 <= k < KS:
                    wsl = weight[:, :, k, l0 + l2:l0 + l2 + 64:2]  # [i,o,p]
                    wsarr[:, dj, :, c, l2, :] = wsl.transpose(2, 0, 1)
    ws = (wsarr.reshape(NWG, WGRP, 128, NCH * 128)
          .transpose(0, 2, 1, 3)
          .reshape(NWG, 128, WGRP * NCH * 128))

    # bs[0, p*128 + l2*64 + o] = bias[o, l0+2p+l2]
    bs = np.ascontiguousarray(
        bias[:, l0:l0 + LS].reshape(CO, NPAIR, 2).transpose(1, 2, 0)
    ).reshape(1, NPAIR * 128)

    return {
        "xs": xs.astype(bfloat16),
        "ws": ws.astype(bfloat16),
        "bs": bs.astype(bfloat16),
    }


def kernel(x, weight, bias):
    global LAST_RESULT
    x = np.asarray(x, np.float32)
    weight = np.asarray(weight, np.float32)
    bias = np.asarray(bias, np.float32)

    nc = _build_nc()

    xpad = np.zeros((B, CI, S + 2 * PAD), np.float32)
    xpad[:, :, PAD:PAD + S] = x

    in_maps = [_prep_core_inputs(xpad, weight, bias, cr)
               for cr in range(NCORES)]

    kw = dict(TRACE_KW)
    if TRACE:
        kw.setdefault("trace", True)
    res = run_bass_kernel_spmd(nc, in_maps, list(range(NCORES)), **kw)
    LAST_RESULT = res

    out = np.empty((B, CO, L), np.float32)
    for cr in range(NCORES):
        r = np.asarray(res.results[cr]["out"], np.float32)   # [64, 4096]
        out[:, :, LS * cr:LS * (cr + 1)] = (
            r.reshape(B, NPAIR, 2, CO).transpose(0, 3, 1, 2).reshape(B, CO, LS)
        )
    return out


# revision 8
# speedup vs baseline: 1.4374x; 1.4374x over previous
"""LocallyConnected1d Trainium2 kernel (8 NeuronCores, sequence-parallel).

Problem: out[b,o,l] = sum_{i,k} xpad[b,i,l+k] * w[i,o,k,l] + bias[o,l]
  B=64, Ci=Co=64, S=L=512, K=9, pad=4.

Strategy:
  * Shard out_seq_len L=512 across 8 cores (64 positions each) so the 75MB
    per-position weight tensor is moved from HBM exactly once (weight DMA is
    the roofline: ~4.7MB/core in bf16).
  * Per core, process positions in pairs (l, l+1). Contract dim is laid out
    as r = dj*64 + i (dj in {0,1}), split into 5 chunks c, where chunk c
    covers window offsets j = 2c+dj of the padded input.
  * matmul per (pair, chunk): stationary lhsT = weight block
    [128=(dj,i), 128=(l2,o)] (full 128-col stationary -> FWL fast weight
    load with bf16), moving rhs = x block [128=(dj,i), 64=b], PSUM
    out [128=(l2,o), 64=b] accumulates over the 5 chunks.
    Weight entry at (dj,i),(l2,o) of chunk c is w[i,o,2c+dj-l2, l+l2]
    (zero if k=2c+dj-l2 outside [0,9)).
  * bias + PSUM->SBUF eviction fused into one DVE tensor_scalar_add
    (bias varies along PSUM partitions -> per-partition scalar operand).
  * All matmul operands bf16 (halves DMA, enables FWL), PSUM fp32,
    bias fp32, output fp32.
  * Dummy warm-up matmuls on the early-arriving x tile keep the PE busy
    while weights stream, flipping the HAM clock gate to full rate.
"""

import sys

sys.path.insert(0, "/opt/trn_rl_repo")

import numpy as np
from ml_dtypes import bfloat16

import concourse.bass as bass
import concourse.bacc as bacc
import concourse.mybir as mybir
from concourse import tile
from concourse.bass_utils import run_bass_kernel_spmd

B = 64
CI = 64
CO = 64
S = 512
KS = 9
PAD = 4
L = 512
NCORES = 8
LS = L // NCORES          # 64 output positions per core
NPAIR = LS // 2           # 32 position pairs per core
NCH = 5                   # contract chunks per pair (j window of 10 -> 5x128)
NT = LS // 2 + NCH - 1    # 36 x-blocks of [128, 64]
OUT_GROUPS = 4            # output DMA granularity (8 pairs each)
WSIZES = [1, 1, 2, 4, 6, 6, 6, 6]   # pairs per weight DMA (ramp-up first)
N_WARMUP = 0             # dummy PE warm-up matmuls

TRACE = False
TRACE_KW: dict = {}
LAST_RESULT = None

_cached_nc = None


def _build_nc():
    global _cached_nc
    if _cached_nc is not None:
        return _cached_nc

    nc = bacc.Bacc("TRN2", target_bir_lowering=False, debug=False,
                   num_devices=NCORES)
    bf = mybir.dt.bfloat16
    f32 = mybir.dt.float32

    xs_d = nc.dram_tensor("xs", [128, NT * 64], bf, kind="ExternalInput").ap()
    ws_d = nc.dram_tensor("ws", [128, NPAIR * NCH * 128], bf,
                          kind="ExternalInput").ap()
    bs_d = nc.dram_tensor("bs", [128, NPAIR], f32, kind="ExternalInput").ap()
    out_d = nc.dram_tensor("out", [128, NPAIR * 64], f32,
                           kind="ExternalOutput").ap()

    with tile.TileContext(nc) as tc:
        with (
            tc.tile_pool(name="xp", bufs=1) as xp,
            tc.tile_pool(name="wp", bufs=len(WSIZES)) as wp,
            tc.tile_pool(name="pp", bufs=5, space="PSUM") as pp,
            tc.tile_pool(name="wu", bufs=1, space="PSUM") as wu,
            tc.tile_pool(name="op", bufs=OUT_GROUPS) as op,
        ):
            xs_t = xp.tile([128, NT * 64], bf, tag="xs")
            nc.sync.dma_start(xs_t[:], xs_d[:])
            bs_t = xp.tile([128, NPAIR], f32, tag="bs")
            nc.scalar.dma_start(bs_t[:], bs_d[:])

            # Weight DMAs: small groups first so the PE can start early,
            # alternating the two HWDGE rings to parallelize issue.
            w_tiles = []
            w_start = []
            c0 = 0
            for g, gsz in enumerate(WSIZES):
                wt = wp.tile([128, gsz * NCH * 128], bf, tag="wt",
                             name=f"wt{g}")
                eng = nc.sync if g % 2 == 0 else nc.scalar
                eng.dma_start(wt[:], ws_d[:, c0 * NCH * 128:
                                          (c0 + gsz) * NCH * 128])
                w_tiles.append(wt)
                w_start.append(c0)
                c0 += gsz
            pair_group = []
            for g, gsz in enumerate(WSIZES):
                pair_group += [g] * gsz

            def w_slice(p, c):
                g = pair_group[p]
                off = ((p - w_start[g]) * NCH + c) * 128
                return w_tiles[g][:, off:off + 128]

            # PE warm-up: harmless matmuls on the x tile (arrives first);
            # keeps the PE busy >3.4us so the HAM clock gate opens before
            # the real accumulation chains begin.
            wu_ps = wu.tile([64, 64], f32, tag="wups")
            for _ in range(N_WARMUP):
                nc.tensor.matmul(wu_ps[:], xs_t[:, 0:64], xs_t[:, 64:128],
                                 start=True, stop=True)

            ppg = NPAIR // OUT_GROUPS
            out_tiles = [op.tile([128, ppg * 64], f32, tag="ot",
                                 name=f"ot{g}")
                         for g in range(OUT_GROUPS)]

            for p in range(NPAIR):
                ps = pp.tile([128, 64], f32, tag="ps", name=f"ps{p}")
                for c in range(NCH):
                    nc.tensor.matmul(
                        ps[:],
                        w_slice(p, c),
                        xs_t[:, (p + c) * 64:(p + c + 1) * 64],
                        start=(c == 0),
                        stop=(c == NCH - 1),
                    )
                ot = out_tiles[p // ppg]
                pp_i = p % ppg
                nc.vector.tensor_scalar_add(
                    ot[:, pp_i * 64:(pp_i + 1) * 64], ps[:], bs_t[:, p:p + 1])

            for g in range(OUT_GROUPS):
                nc.scalar.dma_start(
                    out_d[:, g * ppg * 64:(g + 1) * ppg * 64],
                    out_tiles[g][:])

    nc.compile()
    _cached_nc = nc
    return nc


def _prep_core_inputs(xpad, weight, bias, cr):
    l0 = LS * cr
    # xs[dj*64+i, t*64+b] = xpad[b, i, l0+2t+dj]
    xsl = xpad[:, :, l0:l0 + 2 * NT]                       # [b, i, 72]
    xs = np.ascontiguousarray(
        xsl.reshape(B, CI, NT, 2).transpose(3, 1, 2, 0)    # [dj, i, t, b]
    ).reshape(128, NT * 64)

    # ws[dj*64+i, (p*NCH+c)*128 + l2*64 + o] = w[i,o,2c+dj-l2, l0+2p+l2]
    wsarr = np.zeros((NPAIR, 2, CI, NCH, 2, CO), np.float32)
    for c in range(NCH):
        for dj in range(2):
            for l2 in range(2):
                k = 2 * c + dj - l2
                if 0 <= k < KS:
                    wsl = weight[:, :, k, l0 + l2:l0 + l2 + 64:2]  # [i,o,p]
                    wsarr[:, dj, :, c, l2, :] = wsl.transpose(2, 0, 1)
    ws = np.ascontiguousarray(
        wsarr.transpose(1, 2, 0, 3, 4, 5)        # [dj, i, p, c, l2, o]
    ).reshape(128, NPAIR * NCH * 128)

    # bs[l2*64+o, p] = bias[o, l0+2p+l2]
    bs = np.ascontiguousarray(
        bias[:, l0:l0 + LS].reshape(CO, NPAIR, 2).transpose(2, 0, 1)
    ).reshape(128, NPAIR)

    return {
        "xs": xs.astype(bfloat16),
        "ws": ws.astype(bfloat16),
        "bs": bs.astype(np.float32),
    }


def kernel(x, weight, bias):
    global LAST_RESULT
    x = np.asarray(x, np.float32)
    weight = np.asarray(weight, np.float32)
    bias = np.asarray(bias, np.float32)

    nc = _build_nc()

    xpad = np.zeros((B, CI, S + 2 * PAD), np.float32)
    xpad[:, :, PAD:PAD + S] = x

    in_maps = [_prep_core_inputs(xpad, weight, bias, cr)
               for cr in range(NCORES)]

    kw = dict(TRACE_KW)
    if TRACE:
        kw.setdefault("trace", True)
    res = run_bass_kernel_spmd(nc, in_maps, list(range(NCORES)), **kw)
    LAST_RESULT = res

    out = np.empty((B, CO, L), np.float32)
    for cr in range(NCORES):
        r = np.asarray(res.results[cr]["out"], np.float32)   # [128, 2048]
        out[:, :, LS * cr:LS * (cr + 1)] = (
            r.reshape(2, CO, NPAIR, B).transpose(3, 1, 2, 0).reshape(B, CO, LS)
        )
    return out


# revision 12
# speedup vs baseline: 1.4417x; 1.0030x over previous
"""LocallyConnected1d Trainium2 kernel (8 NeuronCores, sequence-parallel).

Problem: out[b,o,l] = sum_{i,k} xpad[b,i,l+k] * w[i,o,k,l] + bias[o,l]
  B=64, Ci=Co=64, S=L=512, K=9, pad=4.

Strategy:
  * Shard out_seq_len L=512 across 8 cores (64 positions each) so the 75MB
    per-position weight tensor is moved from HBM exactly once (weight DMA is
    the roofline: ~4.7MB/core in bf16).
  * Per core, process positions in pairs (l, l+1). Contract dim is laid out
    as r = dj*64 + i (dj in {0,1}), split into 5 chunks c, where chunk c
    covers window offsets j = 2c+dj of the padded input.
  * matmul per (pair, chunk): stationary lhsT = weight block
    [128=(dj,i), 128=(l2,o)] (full 128-col stationary -> FWL fast weight
    load with bf16), moving rhs = x block [128=(dj,i), 64=b], PSUM
    out [128=(l2,o), 64=b] accumulates over the 5 chunks.
    Weight entry at (dj,i),(l2,o) of chunk c is w[i,o,2c+dj-l2, l+l2]
    (zero if k=2c+dj-l2 outside [0,9)).
  * bias + PSUM->SBUF eviction fused into one DVE tensor_scalar_add
    (bias varies along PSUM partitions -> per-partition scalar operand).
  * All matmul operands bf16 (halves DMA, enables FWL), PSUM fp32,
    bias fp32, output fp32.
  * Dummy warm-up matmuls on the early-arriving x tile keep the PE busy
    while weights stream, flipping the HAM clock gate to full rate.
"""

import sys

sys.path.insert(0, "/opt/trn_rl_repo")

import numpy as np
from ml_dtypes import bfloat16

import concourse.bass as bass
import concourse.bacc as bacc
import concourse.mybir as mybir
from concourse import tile
from concourse.bass_utils import run_bass_kernel_spmd

B = 64
CI = 64
CO = 64
S = 512
KS = 9
PAD = 4
L = 512
NCORES = 8
LS = L // NCORES          # 64 output positions per core
NPAIR = LS // 2           # 32 position pairs per core
NCH = 5                   # contract chunks per pair (j window of 10 -> 5x128)
NT = LS // 2 + NCH - 1    # 36 x-blocks of [128, 64]
OUT_GROUPS = 2            # output DMA granularity (16 pairs each)
WSIZES = [1, 2, 4, 8, 8, 9]   # pairs per weight DMA (ramp-up first)
N_WARMUP = 0             # dummy PE warm-up matmuls

TRACE = False
TRACE_KW: dict = {}
LAST_RESULT = None

_cached_nc = None


def _build_nc():
    global _cached_nc
    if _cached_nc is not None:
        return _cached_nc

    nc = bacc.Bacc("TRN2", target_bir_lowering=False, debug=False,
                   num_devices=NCORES)
    bf = mybir.dt.bfloat16
    f32 = mybir.dt.float32

    xs_d = nc.dram_tensor("xs", [128, NT * 64], bf, kind="ExternalInput").ap()
    # Weights stored group-contiguous in HBM: each DMA reads one fully
    # sequential block (best HBM/row-buffer behavior).
    ws_d = nc.dram_tensor("ws", [128 * NPAIR * NCH * 128], bf,
                          kind="ExternalInput").ap()
    bs_d = nc.dram_tensor("bs", [128, NPAIR], f32, kind="ExternalInput").ap()
    out_d = nc.dram_tensor("out", [128, NPAIR * 64], f32,
                           kind="ExternalOutput").ap()

    with tile.TileContext(nc) as tc:
        with (
            tc.tile_pool(name="xp", bufs=1) as xp,
            tc.tile_pool(name="wp", bufs=len(WSIZES)) as wp,
            tc.tile_pool(name="pp", bufs=5, space="PSUM") as pp,
            tc.tile_pool(name="wu", bufs=1, space="PSUM") as wu,
            tc.tile_pool(name="op", bufs=OUT_GROUPS) as op,
        ):
            xs_t = xp.tile([128, NT * 64], bf, tag="xs")
            nc.sync.dma_start(xs_t[:], xs_d[:])
            bs_t = xp.tile([128, NPAIR], f32, tag="bs")
            nc.scalar.dma_start(bs_t[:], bs_d[:])

            # Weight DMAs: small groups first so the PE can start early,
            # alternating the two HWDGE rings to parallelize issue.
            w_tiles = []
            w_start = []
            c0 = 0
            for g, gsz in enumerate(WSIZES):
                wt = wp.tile([128, gsz * NCH * 128], bf, tag="wt",
                             name=f"wt{g}")
                eng = nc.sync if g % 2 == 0 else nc.scalar
                src = ws_d[c0 * 128 * NCH * 128:
                           (c0 + gsz) * 128 * NCH * 128]
                src = src.rearrange("(p m) -> p m", p=128)
                eng.dma_start(wt[:], src)
                w_tiles.append(wt)
                w_start.append(c0)
                c0 += gsz
            pair_group = []
            for g, gsz in enumerate(WSIZES):
                pair_group += [g] * gsz

            def w_slice(p, c):
                g = pair_group[p]
                off = ((p - w_start[g]) * NCH + c) * 128
                return w_tiles[g][:, off:off + 128]

            # PE warm-up: harmless matmuls on the x tile (arrives first);
            # keeps the PE busy >3.4us so the HAM clock gate opens before
            # the real accumulation chains begin.
            wu_ps = wu.tile([64, 64], f32, tag="wups")
            for _ in range(N_WARMUP):
                nc.tensor.matmul(wu_ps[:], xs_t[:, 0:64], xs_t[:, 64:128],
                                 start=True, stop=True)

            ppg = NPAIR // OUT_GROUPS
            out_tiles = [op.tile([128, ppg * 64], f32, tag="ot",
                                 name=f"ot{g}")
                         for g in range(OUT_GROUPS)]

            for p in range(NPAIR):
                ps = pp.tile([128, 64], f32, tag="ps", name=f"ps{p}")
                for c in range(NCH):
                    nc.tensor.matmul(
                        ps[:],
                        w_slice(p, c),
                        xs_t[:, (p + c) * 64:(p + c + 1) * 64],
                        start=(c == 0),
                        stop=(c == NCH - 1),
                    )
                ot = out_tiles[p // ppg]
                pp_i = p % ppg
                nc.vector.tensor_scalar_add(
                    ot[:, pp_i * 64:(pp_i + 1) * 64], ps[:], bs_t[:, p:p + 1])

            for g in range(OUT_GROUPS):
                nc.scalar.dma_start(
                    out_d[:, g * ppg * 64:(g + 1) * ppg * 64],
                    out_tiles[g][:])

    nc.compile()
    _cached_nc = nc
    return nc


def _prep_core_inputs(xpad, weight, bias, cr):
    l0 = LS * cr
    # xs[dj*64+i, t*64+b] = xpad[b, i, l0+2t+dj]
    xsl = xpad[:, :, l0:l0 + 2 * NT]                       # [b, i, 72]
    xs = np.ascontiguousarray(
        xsl.reshape(B, CI, NT, 2).transpose(3, 1, 2, 0)    # [dj, i, t, b]
    ).reshape(128, NT * 64)

    # ws[dj*64+i, (p*NCH+c)*128 + l2*64 + o] = w[i,o,2c+dj-l2, l0+2p+l2]
    wsarr = np.zeros((NPAIR, 2, CI, NCH, 2, CO), np.float32)
    for c in range(NCH):
        for dj in range(2):
            for l2 in range(2):
                k = 2 * c + dj - l2
                if 0 <= k < KS:
                    wsl = weight[:, :, k, l0 + l2:l0 + l2 + 64:2]  # [i,o,p]
                    wsarr[:, dj, :, c, l2, :] = wsl.transpose(2, 0, 1)
    ws_rows = np.ascontiguousarray(
        wsarr.transpose(1, 2, 0, 3, 4, 5)        # [dj, i, p, c, l2, o]
    ).reshape(128, NPAIR * NCH * 128)
    # group-major contiguous blocks, each [128, gsz*NCH*128] row-major
    blocks = []
    c0 = 0
    for gsz in WSIZES:
        blocks.append(np.ascontiguousarray(
            ws_rows[:, c0 * NCH * 128:(c0 + gsz) * NCH * 128]).reshape(-1))
        c0 += gsz
    ws = np.concatenate(blocks)

    # bs[l2*64+o, p] = bias[o, l0+2p+l2]
    bs = np.ascontiguousarray(
        bias[:, l0:l0 + LS].reshape(CO, NPAIR, 2).transpose(2, 0, 1)
    ).reshape(128, NPAIR)

    return {
        "xs": xs.astype(bfloat16),
        "ws": ws.astype(bfloat16),
        "bs": bs.astype(np.float32),
    }


def kernel(x, weight, bias):
    global LAST_RESULT
    x = np.asarray(x, np.float32)
    weight = np.asarray(weight, np.float32)
    bias = np.asarray(bias, np.float32)

    nc = _build_nc()

    xpad = np.zeros((B, CI, S + 2 * PAD), np.float32)
    xpad[:, :, PAD:PAD + S] = x

    in_maps = [_prep_core_inputs(xpad, weight, bias, cr)
               for cr in range(NCORES)]

    kw = dict(TRACE_KW)
    if TRACE:
        kw.setdefault("trace", True)
    res = run_bass_kernel_spmd(nc, in_maps, list(range(NCORES)), **kw)
    LAST_RESULT = res

    out = np.empty((B, CO, L), np.float32)
    for cr in range(NCORES):
        r = np.asarray(res.results[cr]["out"], np.float32)   # [128, 2048]
        out[:, :, LS * cr:LS * (cr + 1)] = (
            r.reshape(2, CO, NPAIR, B).transpose(3, 1, 2, 0).reshape(B, CO, LS)
        )
    return out


# revision 15
# speedup vs baseline: 1.4466x; 1.0034x over previous
"""LocallyConnected1d Trainium2 kernel (8 NeuronCores, sequence-parallel).

Problem: out[b,o,l] = sum_{i,k} xpad[b,i,l+k] * w[i,o,k,l] + bias[o,l]
  B=64, Ci=Co=64, S=L=512, K=9, pad=4.

Strategy:
  * Shard out_seq_len L=512 across 8 cores (64 positions each) so the 75MB
    per-position weight tensor is moved from HBM exactly once (weight DMA is
    the roofline: ~4.7MB/core in bf16).
  * Per core, process positions in pairs (l, l+1). Contract dim is laid out
    as r = dj*64 + i (dj in {0,1}), split into 5 chunks c, where chunk c
    covers window offsets j = 2c+dj of the padded input.
  * matmul per (pair, chunk): stationary lhsT = weight block
    [128=(dj,i), 128=(l2,o)] (full 128-col stationary -> FWL fast weight
    load with bf16), moving rhs = x block [128=(dj,i), 64=b], PSUM
    out [128=(l2,o), 64=b] accumulates over the 5 chunks.
    Weight entry at (dj,i),(l2,o) of chunk c is w[i,o,2c+dj-l2, l+l2]
    (zero if k=2c+dj-l2 outside [0,9)).
  * bias + PSUM->SBUF eviction fused into one DVE tensor_scalar_add
    (bias varies along PSUM partitions -> per-partition scalar operand).
  * All matmul operands bf16 (halves DMA, enables FWL), PSUM fp32,
    bias fp32, output fp32.
  * Dummy warm-up matmuls on the early-arriving x tile keep the PE busy
    while weights stream, flipping the HAM clock gate to full rate.
"""

import sys

sys.path.insert(0, "/opt/trn_rl_repo")

import numpy as np
from ml_dtypes import bfloat16

import concourse.bass as bass
import concourse.bacc as bacc
import concourse.mybir as mybir
from concourse import tile
from concourse.bass_utils import run_bass_kernel_spmd

B = 64
CI = 64
CO = 64
S = 512
KS = 9
PAD = 4
L = 512
NCORES = 8
LS = L // NCORES          # 64 output positions per core
NPAIR = LS // 2           # 32 position pairs per core
NCH = 5                   # contract chunks per pair (j window of 10 -> 5x128)
NT = LS // 2 + NCH - 1    # 36 x-blocks of [128, 64]
OUT_GROUPS = 4            # output DMA granularity (8 pairs each)
WSIZES = [1, 2, 4, 8, 9, 8]   # pairs per weight DMA (ramp-up first)
N_WARMUP = 0             # dummy PE warm-up matmuls

TRACE = False
TRACE_KW: dict = {}
LAST_RESULT = None

_cached_nc = None


def _build_nc():
    global _cached_nc
    if _cached_nc is not None:
        return _cached_nc

    nc = bacc.Bacc("TRN2", target_bir_lowering=False, debug=False,
                   num_devices=NCORES)
    bf = mybir.dt.bfloat16
    f32 = mybir.dt.float32

    xs_d = nc.dram_tensor("xs", [128, NT * 64], bf, kind="ExternalInput").ap()
    # Weights stored group-contiguous in HBM: each DMA reads one fully
    # sequential block (best HBM/row-buffer behavior).
    ws_d = nc.dram_tensor("ws", [128 * NPAIR * NCH * 128], bf,
                          kind="ExternalInput").ap()
    bs_d = nc.dram_tensor("bs", [128, NPAIR], f32, kind="ExternalInput").ap()
    out_d = nc.dram_tensor("out", [128, NPAIR * 64], f32,
                           kind="ExternalOutput").ap()

    with tile.TileContext(nc) as tc:
        with (
            tc.tile_pool(name="xp", bufs=1) as xp,
            tc.tile_pool(name="wp", bufs=len(WSIZES)) as wp,
            tc.tile_pool(name="pp", bufs=5, space="PSUM") as pp,
            tc.tile_pool(name="wu", bufs=1, space="PSUM") as wu,
            tc.tile_pool(name="op", bufs=OUT_GROUPS) as op,
        ):
            xs_t = xp.tile([128, NT * 64], bf, tag="xs")
            nc.sync.dma_start(xs_t[:], xs_d[:])
            bs_t = xp.tile([128, NPAIR], f32, tag="bs")
            nc.sync.dma_start(bs_t[:], bs_d[:])

            # Weight DMAs: small groups first so the PE can start early,
            # alternating the two HWDGE rings to parallelize issue.
            w_tiles = []
            w_start = []
            c0 = 0
            for g, gsz in enumerate(WSIZES):
                wt = wp.tile([128, gsz * NCH * 128], bf, tag="wt",
                             name=f"wt{g}")
                eng = nc.sync if g % 2 == 0 else nc.scalar
                src = ws_d[c0 * 128 * NCH * 128:
                           (c0 + gsz) * 128 * NCH * 128]
                src = src.rearrange("(p m) -> p m", p=128)
                eng.dma_start(wt[:], src)
                w_tiles.append(wt)
                w_start.append(c0)
                c0 += gsz
            pair_group = []
            for g, gsz in enumerate(WSIZES):
                pair_group += [g] * gsz

            def w_slice(p, c):
                g = pair_group[p]
                off = ((p - w_start[g]) * NCH + c) * 128
                return w_tiles[g][:, off:off + 128]

            # PE warm-up: harmless matmuls on the x tile (arrives first);
            # keeps the PE busy >3.4us so the HAM clock gate opens before
            # the real accumulation chains begin.
            wu_ps = wu.tile([64, 64], f32, tag="wups")
            for _ in range(N_WARMUP):
                nc.tensor.matmul(wu_ps[:], xs_t[:, 0:64], xs_t[:, 64:128],
                                 start=True, stop=True)

            ppg = NPAIR // OUT_GROUPS
            out_tiles = [op.tile([128, ppg * 64], f32, tag="ot",
                                 name=f"ot{g}")
                         for g in range(OUT_GROUPS)]

            for p in range(NPAIR):
                ps = pp.tile([128, 64], f32, tag="ps", name=f"ps{p}")
                for c in range(NCH):
                    nc.tensor.matmul(
                        ps[:],
                        w_slice(p, c),
                        xs_t[:, (p + c) * 64:(p + c + 1) * 64],
                        start=(c == 0),
                        stop=(c == NCH - 1),
                    )
                ot = out_tiles[p // ppg]
                pp_i = p % ppg
                nc.vector.tensor_scalar_add(
                    ot[:, pp_i * 64:(pp_i + 1) * 64], ps[:], bs_t[:, p:p + 1])

            for g in range(OUT_GROUPS):
                eng = nc.scalar if g % 2 == 0 else nc.sync
                eng.dma_start(
                    out_d[:, g * ppg * 64:(g + 1) * ppg * 64],
                    out_tiles[g][:])

    nc.compile()
    _cached_nc = nc
    return nc


def _prep_core_inputs(xpad, weight, bias, cr):
    l0 = LS * cr
    # xs[dj*64+i, t*64+b] = xpad[b, i, l0+2t+dj]
    xsl = xpad[:, :, l0:l0 + 2 * NT]                       # [b, i, 72]
    xs = np.ascontiguousarray(
        xsl.reshape(B, CI, NT, 2).transpose(3, 1, 2, 0)    # [dj, i, t, b]
    ).reshape(128, NT * 64)

    # ws[dj*64+i, (p*NCH+c)*128 + l2*64 + o] = w[i,o,2c+dj-l2, l0+2p+l2]
    wsarr = np.zeros((NPAIR, 2, CI, NCH, 2, CO), np.float32)
    for c in range(NCH):
        for dj in range(2):
            for l2 in range(2):
                k = 2 * c + dj - l2
                if 0 <= k < KS:
                    wsl = weight[:, :, k, l0 + l2:l0 + l2 + 64:2]  # [i,o,p]
                    wsarr[:, dj, :, c, l2, :] = wsl.transpose(2, 0, 1)
    ws_rows = np.ascontiguousarray(
        wsarr.transpose(1, 2, 0, 3, 4, 5)        # [dj, i, p, c, l2, o]
    ).reshape(128, NPAIR * NCH * 128)
    # group-major contiguous blocks, each [128, gsz*NCH*128] row-major
    blocks = []
    c0 = 0
    for gsz in WSIZES:
        blocks.append(np.ascontiguousarray(
            ws_rows[:, c0 * NCH * 128:(c0 + gsz) * NCH * 128]).reshape(-1))
        c0 += gsz
    ws = np.concatenate(blocks)

    # bs[l2*64+o, p] = bias[o, l0+2p+l2]
    bs = np.ascontiguousarray(
        bias[:, l0:l0 + LS].reshape(CO, NPAIR, 2).transpose(2, 0, 1)
    ).reshape(128, NPAIR)

    return {
        "xs": xs.astype(bfloat16),
        "ws": ws.astype(bfloat16),
        "bs": bs.astype(np.float32),
    }


def kernel(x, weight, bias):
    global LAST_RESULT
    x = np.asarray(x, np.float32)
    weight = np.asarray(weight, np.float32)
    bias = np.asarray(bias, np.float32)

    nc = _build_nc()

    xpad = np.zeros((B, CI, S + 2 * PAD), np.float32)
    xpad[:, :, PAD:PAD + S] = x

    in_maps = [_prep_core_inputs(xpad, weight, bias, cr)
               for cr in range(NCORES)]

    kw = dict(TRACE_KW)
    if TRACE:
        kw.setdefault("trace", True)
    res = run_bass_kernel_spmd(nc, in_maps, list(range(NCORES)), **kw)
    LAST_RESULT = res

    out = np.empty((B, CO, L), np.float32)
    for cr in range(NCORES):
        r = np.asarray(res.results[cr]["out"], np.float32)   # [128, 2048]
        out[:, :, LS * cr:LS * (cr + 1)] = (
            r.reshape(2, CO, NPAIR, B).transpose(3, 1, 2, 0).reshape(B, CO, LS)
        )
    return out


# revision 16
# speedup vs baseline: 1.4859x; 1.0272x over previous
"""LocallyConnected1d Trainium2 kernel (8 NeuronCores, sequence-parallel).

Problem: out[b,o,l] = sum_{i,k} xpad[b,i,l+k] * w[i,o,k,l] + bias[o,l]
  B=64, Ci=Co=64, S=L=512, K=9, pad=4.

Strategy:
  * Shard out_seq_len L=512 across 8 cores (64 positions each) so the 75MB
    per-position weight tensor is moved from HBM exactly once (weight DMA is
    the roofline: ~4.7MB/core in bf16).
  * Per core, process positions in pairs (l, l+1). Contract dim is laid out
    as r = dj*64 + i (dj in {0,1}), split into 5 chunks c, where chunk c
    covers window offsets j = 2c+dj of the padded input.
  * matmul per (pair, chunk): stationary lhsT = weight block
    [128=(dj,i), 128=(l2,o)] (full 128-col stationary -> FWL fast weight
    load with bf16), moving rhs = x block [128=(dj,i), 64=b], PSUM
    out [128=(l2,o), 64=b] accumulates over the 5 chunks.
    Weight entry at (dj,i),(l2,o) of chunk c is w[i,o,2c+dj-l2, l+l2]
    (zero if k=2c+dj-l2 outside [0,9)).
  * bias + PSUM->SBUF eviction fused into one DVE tensor_scalar_add
    (bias varies along PSUM partitions -> per-partition scalar operand).
  * All matmul operands bf16 (halves DMA, enables FWL), PSUM fp32,
    bias fp32, output fp32.
  * Dummy warm-up matmuls on the early-arriving x tile keep the PE busy
    while weights stream, flipping the HAM clock gate to full rate.
"""

import sys

sys.path.insert(0, "/opt/trn_rl_repo")

import numpy as np
from ml_dtypes import bfloat16

import concourse.bass as bass
import concourse.bacc as bacc
import concourse.mybir as mybir
from concourse import tile
from concourse.bass_utils import run_bass_kernel_spmd

B = 64
CI = 64
CO = 64
S = 512
KS = 9
PAD = 4
L = 512
NCORES = 8
LS = L // NCORES          # 64 output positions per core
NPAIR = LS // 2           # 32 position pairs per core
NCH = 5                   # contract chunks per pair (j window of 10 -> 5x128)
NT = LS // 2 + NCH - 1    # 36 x-blocks of [128, 64]
OUT_GROUPS = 4            # output DMA granularity (8 pairs each)
WGRP = 2                  # pairs per weight DMA; groups alternate HWDGE rings
NWG = NPAIR // WGRP
WSIZES = [WGRP] * NWG
N_WARMUP = 0             # dummy PE warm-up matmuls

TRACE = False
TRACE_KW: dict = {}
LAST_RESULT = None

_cached_nc = None


def _build_nc():
    global _cached_nc
    if _cached_nc is not None:
        return _cached_nc

    nc = bacc.Bacc("TRN2", target_bir_lowering=False, debug=False,
                   num_devices=NCORES)
    bf = mybir.dt.bfloat16
    f32 = mybir.dt.float32

    xs_d = nc.dram_tensor("xs", [128, NT * 64], bf, kind="ExternalInput").ap()
    # Weights stored group-contiguous in HBM: each DMA reads one fully
    # sequential block (best HBM/row-buffer behavior).
    ws_d = nc.dram_tensor("ws", [128 * NPAIR * NCH * 128], bf,
                          kind="ExternalInput").ap()
    bs_d = nc.dram_tensor("bs", [128, NPAIR], f32, kind="ExternalInput").ap()
    out_d = nc.dram_tensor("out", [128, NPAIR * 64], bf,
                           kind="ExternalOutput").ap()

    with tile.TileContext(nc) as tc:
        with (
            tc.tile_pool(name="xp", bufs=1) as xp,
            tc.tile_pool(name="wp", bufs=len(WSIZES)) as wp,
            tc.tile_pool(name="pp", bufs=5, space="PSUM") as pp,
            tc.tile_pool(name="wu", bufs=1, space="PSUM") as wu,
            tc.tile_pool(name="op", bufs=OUT_GROUPS) as op,
        ):
            xs_t = xp.tile([128, NT * 64], bf, tag="xs")
            nc.sync.dma_start(xs_t[:], xs_d[:])
            bs_t = xp.tile([128, NPAIR], f32, tag="bs")
            nc.scalar.dma_start(bs_t[:], bs_d[:])

            # Weight DMAs: small groups first so the PE can start early,
            # alternating the two HWDGE rings to parallelize issue.
            w_tiles = []
            w_start = []
            c0 = 0
            for g, gsz in enumerate(WSIZES):
                wt = wp.tile([128, gsz * NCH * 128], bf, tag="wt",
                             name=f"wt{g}")
                eng = nc.sync if g % 2 == 0 else nc.scalar
                src = ws_d[c0 * 128 * NCH * 128:
                           (c0 + gsz) * 128 * NCH * 128]
                src = src.rearrange("(p m) -> p m", p=128)
                eng.dma_start(wt[:], src)
                w_tiles.append(wt)
                w_start.append(c0)
                c0 += gsz
            pair_group = []
            for g, gsz in enumerate(WSIZES):
                pair_group += [g] * gsz

            def w_slice(p, c):
                g = pair_group[p]
                off = ((p - w_start[g]) * NCH + c) * 128
                return w_tiles[g][:, off:off + 128]

            # PE warm-up: harmless matmuls on the x tile (arrives first);
            # keeps the PE busy >3.4us so the HAM clock gate opens before
            # the real accumulation chains begin.
            wu_ps = wu.tile([64, 64], f32, tag="wups")
            for _ in range(N_WARMUP):
                nc.tensor.matmul(wu_ps[:], xs_t[:, 0:64], xs_t[:, 64:128],
                                 start=True, stop=True)

            ppg = NPAIR // OUT_GROUPS
            out_tiles = [op.tile([128, ppg * 64], bf, tag="ot",
                                 name=f"ot{g}")
                         for g in range(OUT_GROUPS)]

            for p in range(NPAIR):
                ps = pp.tile([128, 64], f32, tag="ps", name=f"ps{p}")
                for c in range(NCH):
                    nc.tensor.matmul(
                        ps[:],
                        w_slice(p, c),
                        xs_t[:, (p + c) * 64:(p + c + 1) * 64],
                        start=(c == 0),
                        stop=(c == NCH - 1),
                    )
                ot = out_tiles[p // ppg]
                pp_i = p % ppg
                nc.vector.tensor_scalar_add(
                    ot[:, pp_i * 64:(pp_i + 1) * 64], ps[:], bs_t[:, p:p + 1])

            for g in range(OUT_GROUPS):
                eng = nc.scalar if g % 2 == 0 else nc.sync
                eng.dma_start(
                    out_d[:, g * ppg * 64:(g + 1) * ppg * 64],
                    out_tiles[g][:])

    nc.compile()
    _cached_nc = nc
    return nc


def _prep_core_inputs(xpad, weight, bias, cr):
    l0 = LS * cr
    # xs[dj*64+i, t*64+b] = xpad[b, i, l0+2t+dj]
    xsl = xpad[:, :, l0:l0 + 2 * NT]                       # [b, i, 72]
    xs = np.ascontiguousarray(
        xsl.reshape(B, CI, NT, 2).transpose(3, 1, 2, 0)    # [dj, i, t, b]
    ).reshape(128, NT * 64)

    # ws[dj*64+i, (p*NCH+c)*128 + l2*64 + o] = w[i,o,2c+dj-l2, l0+2p+l2]
    wsarr = np.zeros((NPAIR, 2, CI, NCH, 2, CO), np.float32)
    for c in range(NCH):
        for dj in range(2):
            for l2 in range(2):
                k = 2 * c + dj - l2
                if 0 <= k < KS:
                    wsl = weight[:, :, k, l0 + l2:l0 + l2 + 64:2]  # [i,o,p]
                    wsarr[:, dj, :, c, l2, :] = wsl.transpose(2, 0, 1)
    ws_rows = np.ascontiguousarray(
        wsarr.transpose(1, 2, 0, 3, 4, 5)        # [dj, i, p, c, l2, o]
    ).reshape(128, NPAIR * NCH * 128)
    # group-major contiguous blocks, each [128, gsz*NCH*128] row-major
    blocks = []
    c0 = 0
    for gsz in WSIZES:
        blocks.append(np.ascontiguousarray(
            ws_rows[:, c0 * NCH * 128:(c0 + gsz) * NCH * 128]).reshape(-1))
        c0 += gsz
    ws = np.concatenate(blocks)

    # bs[l2*64+o, p] = bias[o, l0+2p+l2]
    bs = np.ascontiguousarray(
        bias[:, l0:l0 + LS].reshape(CO, NPAIR, 2).transpose(2, 0, 1)
    ).reshape(128, NPAIR)

    return {
        "xs": xs.astype(bfloat16),
        "ws": ws.astype(bfloat16),
        "bs": bs.astype(np.float32),
    }


def kernel(x, weight, bias):
    global LAST_RESULT
    x = np.asarray(x, np.float32)
    weight = np.asarray(weight, np.float32)
    bias = np.asarray(bias, np.float32)

    nc = _build_nc()

    xpad = np.zeros((B, CI, S + 2 * PAD), np.float32)
    xpad[:, :, PAD:PAD + S] = x

    in_maps = [_prep_core_inputs(xpad, weight, bias, cr)
               for cr in range(NCORES)]

    kw = dict(TRACE_KW)
    if TRACE:
        kw.setdefault("trace", True)
    res = run_bass_kernel_spmd(nc, in_maps, list(range(NCORES)), **kw)
    LAST_RESULT = res

    out = np.empty((B, CO, L), np.float32)
    for cr in range(NCORES):
        r = np.asarray(res.results[cr]["out"]).astype(np.float32)  # [128, 2048]
        out[:, :, LS * cr:LS * (cr + 1)] = (
            r.reshape(2, CO, NPAIR, B).transpose(3, 1, 2, 0).reshape(B, CO, LS)
        )
    return out


# revision 17
# speedup vs baseline: 1.5086x; 1.0153x over previous
"""LocallyConnected1d Trainium2 kernel (8 NeuronCores, sequence-parallel).

Problem: out[b,o,l] = sum_{i,k} xpad[b,i,l+k] * w[i,o,k,l] + bias[o,l]
  B=64, Ci=Co=64, S=L=512, K=9, pad=4.

Strategy:
  * Shard out_seq_len L=512 across 8 cores (64 positions each) so the 75MB
    per-position weight tensor is moved from HBM exactly once (weight DMA is
    the roofline: ~4.7MB/core in bf16).
  * Per core, process positions in pairs (l, l+1). Contract dim is laid out
    as r = dj*64 + i (dj in {0,1}), split into 5 chunks c, where chunk c
    covers window offsets j = 2c+dj of the padded input.
  * matmul per (pair, chunk): stationary lhsT = weight block
    [128=(dj,i), 128=(l2,o)] (full 128-col stationary -> FWL fast weight
    load with bf16), moving rhs = x block [128=(dj,i), 64=b], PSUM
    out [128=(l2,o), 64=b] accumulates over the 5 chunks.
    Weight entry at (dj,i),(l2,o) of chunk c is w[i,o,2c+dj-l2, l+l2]
    (zero if k=2c+dj-l2 outside [0,9)).
  * bias + PSUM->SBUF eviction fused into one DVE tensor_scalar_add
    (bias varies along PSUM partitions -> per-partition scalar operand).
  * All matmul operands bf16 (halves DMA, enables FWL), PSUM fp32,
    bias fp32, output fp32.
  * Dummy warm-up matmuls on the early-arriving x tile keep the PE busy
    while weights stream, flipping the HAM clock gate to full rate.
"""

import sys

sys.path.insert(0, "/opt/trn_rl_repo")

import numpy as np
from ml_dtypes import bfloat16

import concourse.bass as bass
import concourse.bacc as bacc
import concourse.mybir as mybir
from concourse import tile
from concourse.bass_utils import run_bass_kernel_spmd

B = 64
CI = 64
CO = 64
S = 512
KS = 9
PAD = 4
L = 512
NCORES = 8
LS = L // NCORES          # 64 output positions per core
NPAIR = LS // 2           # 32 position pairs per core
NCH = 5                   # contract chunks per pair (j window of 10 -> 5x128)
NT = LS // 2 + NCH - 1    # 36 x-blocks of [128, 64]
OUT_GROUPS = 4            # output DMA granularity (8 pairs each)
WGRP = 2                  # pairs per weight DMA; groups alternate HWDGE rings
NWG = NPAIR // WGRP
WSIZES = [WGRP] * NWG
N_WARMUP = 0             # dummy PE warm-up matmuls

TRACE = False
TRACE_KW: dict = {}
LAST_RESULT = None

_cached_nc = None


def _build_nc():
    global _cached_nc
    if _cached_nc is not None:
        return _cached_nc

    nc = bacc.Bacc("TRN2", target_bir_lowering=False, debug=False,
                   num_devices=NCORES)
    bf = mybir.dt.bfloat16
    f32 = mybir.dt.float32

    xs_da = nc.dram_tensor("xsa", [128, (NT // 2) * 64], bf,
                           kind="ExternalInput").ap()
    xs_db = nc.dram_tensor("xsb", [128, (NT - NT // 2) * 64], bf,
                           kind="ExternalInput").ap()
    # Weights stored group-contiguous in HBM: each DMA reads one fully
    # sequential block (best HBM/row-buffer behavior).
    ws_d = nc.dram_tensor("ws", [128 * NPAIR * NCH * 128], bf,
                          kind="ExternalInput").ap()
    bs_d = nc.dram_tensor("bs", [128, NPAIR], f32, kind="ExternalInput").ap()
    out_d = nc.dram_tensor("out", [128, NPAIR * 64], bf,
                           kind="ExternalOutput").ap()

    with tile.TileContext(nc) as tc:
        with (
            tc.tile_pool(name="xp", bufs=1) as xp,
            tc.tile_pool(name="wp", bufs=len(WSIZES)) as wp,
            tc.tile_pool(name="pp", bufs=5, space="PSUM") as pp,
            tc.tile_pool(name="wu", bufs=1, space="PSUM") as wu,
            tc.tile_pool(name="op", bufs=OUT_GROUPS) as op,
        ):
            xs_ta = xp.tile([128, (NT // 2) * 64], bf, tag="xsa")
            nc.sync.dma_start(xs_ta[:], xs_da[:])
            xs_tb = xp.tile([128, (NT - NT // 2) * 64], bf, tag="xsb")
            nc.scalar.dma_start(xs_tb[:], xs_db[:])
            bs_t = xp.tile([128, NPAIR], f32, tag="bs")

            def xs_block(t):
                h = NT // 2
                if t < h:
                    return xs_ta[:, t * 64:(t + 1) * 64]
                return xs_tb[:, (t - h) * 64:(t - h + 1) * 64]

            # Weight DMAs: small groups first so the PE can start early,
            # alternating the two HWDGE rings to parallelize issue.
            w_tiles = []
            w_start = []
            c0 = 0
            for g, gsz in enumerate(WSIZES):
                wt = wp.tile([128, gsz * NCH * 128], bf, tag="wt",
                             name=f"wt{g}")
                eng = nc.sync if g % 2 == 0 else nc.scalar
                src = ws_d[c0 * 128 * NCH * 128:
                           (c0 + gsz) * 128 * NCH * 128]
                src = src.rearrange("(p m) -> p m", p=128)
                eng.dma_start(wt[:], src)
                if g == 1:
                    # bias rides after the first scalar-ring weight group:
                    # off the critical path of both the ramp and the drain.
                    nc.scalar.dma_start(bs_t[:], bs_d[:])
                w_tiles.append(wt)
                w_start.append(c0)
                c0 += gsz
            pair_group = []
            for g, gsz in enumerate(WSIZES):
                pair_group += [g] * gsz

            def w_slice(p, c):
                g = pair_group[p]
                off = ((p - w_start[g]) * NCH + c) * 128
                return w_tiles[g][:, off:off + 128]

            # PE warm-up: harmless matmuls on the x tile (arrives first);
            # keeps the PE busy >3.4us so the HAM clock gate opens before
            # the real accumulation chains begin.
            wu_ps = wu.tile([64, 64], f32, tag="wups")
            for _ in range(N_WARMUP):
                nc.tensor.matmul(wu_ps[:], xs_t[:, 0:64], xs_t[:, 64:128],
                                 start=True, stop=True)

            ppg = NPAIR // OUT_GROUPS
            out_tiles = [op.tile([128, ppg * 64], bf, tag="ot",
                                 name=f"ot{g}")
                         for g in range(OUT_GROUPS)]

            for p in range(NPAIR):
                ps = pp.tile([128, 64], f32, tag="ps", name=f"ps{p}")
                for c in range(NCH):
                    nc.tensor.matmul(
                        ps[:],
                        w_slice(p, c),
                        xs_block(p + c),
                        start=(c == 0),
                        stop=(c == NCH - 1),
                    )
                ot = out_tiles[p // ppg]
                pp_i = p % ppg
                nc.vector.tensor_scalar_add(
                    ot[:, pp_i * 64:(pp_i + 1) * 64], ps[:], bs_t[:, p:p + 1])

            for g in range(OUT_GROUPS):
                eng = nc.scalar if g % 2 == 0 else nc.sync
                eng.dma_start(
                    out_d[:, g * ppg * 64:(g + 1) * ppg * 64],
                    out_tiles[g][:])

    nc.compile()
    _cached_nc = nc
    return nc


def _prep_core_inputs(xpad, weight, bias, cr):
    l0 = LS * cr
    # xs[dj*64+i, t*64+b] = xpad[b, i, l0+2t+dj]
    xsl = xpad[:, :, l0:l0 + 2 * NT]                       # [b, i, 72]
    xs = np.ascontiguousarray(
        xsl.reshape(B, CI, NT, 2).transpose(3, 1, 2, 0)    # [dj, i, t, b]
    ).reshape(128, NT * 64)
    h = NT // 2
    xsa = np.ascontiguousarray(xs[:, :h * 64])
    xsb = np.ascontiguousarray(xs[:, h * 64:])

    # ws[dj*64+i, (p*NCH+c)*128 + l2*64 + o] = w[i,o,2c+dj-l2, l0+2p+l2]
    wsarr = np.zeros((NPAIR, 2, CI, NCH, 2, CO), np.float32)
    for c in range(NCH):
        for dj in range(2):
            for l2 in range(2):
                k = 2 * c + dj - l2
                if 0 <= k < KS:
                    wsl = weight[:, :, k, l0 + l2:l0 + l2 + 64:2]  # [i,o,p]
                    wsarr[:, dj, :, c, l2, :] = wsl.transpose(2, 0, 1)
    ws_rows = np.ascontiguousarray(
        wsarr.transpose(1, 2, 0, 3, 4, 5)        # [dj, i, p, c, l2, o]
    ).reshape(128, NPAIR * NCH * 128)
    # group-major contiguous blocks, each [128, gsz*NCH*128] row-major
    blocks = []
    c0 = 0
    for gsz in WSIZES:
        blocks.append(np.ascontiguousarray(
            ws_rows[:, c0 * NCH * 128:(c0 + gsz) * NCH * 128]).reshape(-1))
        c0 += gsz
    ws = np.concatenate(blocks)

    # bs[l2*64+o, p] = bias[o, l0+2p+l2]
    bs = np.ascontiguousarray(
        bias[:, l0:l0 + LS].reshape(CO, NPAIR, 2).transpose(2, 0, 1)
    ).reshape(128, NPAIR)

    return {
        "xsa": xsa.astype(bfloat16),
        "xsb": xsb.astype(bfloat16),
        "ws": ws.astype(bfloat16),
        "bs": bs.astype(np.float32),
    }


def kernel(x, weight, bias):
    global LAST_RESULT
    x = np.asarray(x, np.float32)
    weight = np.asarray(weight, np.float32)
    bias = np.asarray(bias, np.float32)

    nc = _build_nc()

    xpad = np.zeros((B, CI, S + 2 * PAD), np.float32)
    xpad[:, :, PAD:PAD + S] = x

    in_maps = [_prep_core_inputs(xpad, weight, bias, cr)
               for cr in range(NCORES)]

    kw = dict(TRACE_KW)
    if TRACE:
        kw.setdefault("trace", True)
    res = run_bass_kernel_spmd(nc, in_maps, list(range(NCORES)), **kw)
    LAST_RESULT = res

    out = np.empty((B, CO, L), np.float32)
    for cr in range(NCORES):
        r = np.asarray(res.results[cr]["out"]).astype(np.float32)  # [128, 2048]
        out[:, :, LS * cr:LS * (cr + 1)] = (
            r.reshape(2, CO, NPAIR, B).transpose(3, 1, 2, 0).reshape(B, CO, LS)
        )
    return out


# revision 18
# speedup vs baseline: 1.5590x; 1.0334x over previous
"""LocallyConnected1d Trainium2 kernel (8 NeuronCores, sequence-parallel).

Problem: out[b,o,l] = sum_{i,k} xpad[b,i,l+k] * w[i,o,k,l] + bias[o,l]
  B=64, Ci=Co=64, S=L=512, K=9, pad=4.

Strategy:
  * Shard out_seq_len L=512 across 8 cores (64 positions each) so the 75MB
    per-position weight tensor is moved from HBM exactly once (weight DMA is
    the roofline: ~4.7MB/core in bf16).
  * Per core, process positions in pairs (l, l+1). Contract dim is laid out
    as r = dj*64 + i (dj in {0,1}), split into 5 chunks c, where chunk c
    covers window offsets j = 2c+dj of the padded input.
  * matmul per (pair, chunk): stationary lhsT = weight block
    [128=(dj,i), 128=(l2,o)] (full 128-col stationary -> FWL fast weight
    load with bf16), moving rhs = x block [128=(dj,i), 64=b], PSUM
    out [128=(l2,o), 64=b] accumulates over the 5 chunks.
    Weight entry at (dj,i),(l2,o) of chunk c is w[i,o,2c+dj-l2, l+l2]
    (zero if k=2c+dj-l2 outside [0,9)).
  * bias + PSUM->SBUF eviction fused into one DVE tensor_scalar_add
    (bias varies along PSUM partitions -> per-partition scalar operand).
  * All matmul operands bf16 (halves DMA, enables FWL), PSUM fp32,
    bias fp32, output fp32.
  * Dummy warm-up matmuls on the early-arriving x tile keep the PE busy
    while weights stream, flipping the HAM clock gate to full rate.
"""

import sys

sys.path.insert(0, "/opt/trn_rl_repo")

import numpy as np
from ml_dtypes import bfloat16

import concourse.bass as bass
import concourse.bacc as bacc
import concourse.mybir as mybir
from concourse import tile
from concourse.bass_utils import run_bass_kernel_spmd

B = 64
CI = 64
CO = 64
S = 512
KS = 9
PAD = 4
L = 512
NCORES = 8
LS = L // NCORES          # 64 output positions per core
NPAIR = LS // 2           # 32 position pairs per core
NCH = 5                   # contract chunks per pair (j window of 10 -> 5x128)
NT = LS // 2 + NCH - 1    # 36 x-blocks of [128, 64]
OUT_SIZES = [10, 10, 8, 4]   # pairs per output DMA (small last -> short tail)
OUT_GROUPS = len(OUT_SIZES)
WGRP = 4                  # pairs per weight DMA; groups alternate HWDGE rings
NWG = NPAIR // WGRP
WSIZES = [WGRP] * NWG
N_WARMUP = 0             # dummy PE warm-up matmuls

TRACE = False
TRACE_KW: dict = {}
LAST_RESULT = None

_cached_nc = None


def _build_nc():
    global _cached_nc
    if _cached_nc is not None:
        return _cached_nc

    nc = bacc.Bacc("TRN2", target_bir_lowering=False, debug=False,
                   num_devices=NCORES)
    bf = mybir.dt.bfloat16
    f32 = mybir.dt.float32

    xs_da = nc.dram_tensor("xsa", [128, (NT // 2) * 64], bf,
                           kind="ExternalInput").ap()
    xs_db = nc.dram_tensor("xsb", [128, (NT - NT // 2) * 64], bf,
                           kind="ExternalInput").ap()
    # Weights stored group-contiguous in HBM: each DMA reads one fully
    # sequential block (best HBM/row-buffer behavior).
    ws_d = nc.dram_tensor("ws", [128 * NPAIR * NCH * 128], bf,
                          kind="ExternalInput").ap()
    bs_d = nc.dram_tensor("bs", [128, NPAIR], f32, kind="ExternalInput").ap()
    out_d = nc.dram_tensor("out", [128, NPAIR * 64], bf,
                           kind="ExternalOutput").ap()

    with tile.TileContext(nc) as tc:
        with (
            tc.tile_pool(name="xp", bufs=1) as xp,
            tc.tile_pool(name="wp", bufs=len(WSIZES)) as wp,
            tc.tile_pool(name="pp", bufs=5, space="PSUM") as pp,
            tc.tile_pool(name="wu", bufs=1, space="PSUM") as wu,
            tc.tile_pool(name="op", bufs=OUT_GROUPS) as op,
        ):
            xs_ta = xp.tile([128, (NT // 2) * 64], bf, tag="xsa")
            nc.sync.dma_start(xs_ta[:], xs_da[:])
            xs_tb = xp.tile([128, (NT - NT // 2) * 64], bf, tag="xsb")
            nc.scalar.dma_start(xs_tb[:], xs_db[:])
            bs_t = xp.tile([128, NPAIR], f32, tag="bs")

            def xs_block(t):
                h = NT // 2
                if t < h:
                    return xs_ta[:, t * 64:(t + 1) * 64]
                return xs_tb[:, (t - h) * 64:(t - h + 1) * 64]

            # Weight DMAs: small groups first so the PE can start early,
            # alternating the two HWDGE rings to parallelize issue.
            w_tiles = []
            w_start = []
            c0 = 0
            for g, gsz in enumerate(WSIZES):
                wt = wp.tile([128, gsz * NCH * 128], bf, tag="wt",
                             name=f"wt{g}")
                eng = nc.sync if g % 2 == 0 else nc.scalar
                src = ws_d[c0 * 128 * NCH * 128:
                           (c0 + gsz) * 128 * NCH * 128]
                src = src.rearrange("(p m) -> p m", p=128)
                eng.dma_start(wt[:], src)
                if g == 1:
                    # bias rides after the first scalar-ring weight group:
                    # off the critical path of both the ramp and the drain.
                    nc.scalar.dma_start(bs_t[:], bs_d[:])
                w_tiles.append(wt)
                w_start.append(c0)
                c0 += gsz
            pair_group = []
            for g, gsz in enumerate(WSIZES):
                pair_group += [g] * gsz

            def w_slice(p, c):
                g = pair_group[p]
                off = ((p - w_start[g]) * NCH + c) * 128
                return w_tiles[g][:, off:off + 128]

            # PE warm-up: harmless matmuls on the x tile (arrives first);
            # keeps the PE busy >3.4us so the HAM clock gate opens before
            # the real accumulation chains begin.
            wu_ps = wu.tile([64, 64], f32, tag="wups")
            for _ in range(N_WARMUP):
                nc.tensor.matmul(wu_ps[:], xs_t[:, 0:64], xs_t[:, 64:128],
                                 start=True, stop=True)

            out_tiles = [op.tile([128, osz * 64], bf, tag=f"ot{g}",
                                 name=f"ot{g}", bufs=1)
                         for g, osz in enumerate(OUT_SIZES)]
            out_group_of = []
            out_off_of = []
            for g, osz in enumerate(OUT_SIZES):
                for j in range(osz):
                    out_group_of.append(g)
                    out_off_of.append(j)
            out_base = np.cumsum([0] + OUT_SIZES[:-1])

            for p in range(NPAIR):
                ps = pp.tile([128, 64], f32, tag="ps", name=f"ps{p}")
                for c in range(NCH):
                    nc.tensor.matmul(
                        ps[:],
                        w_slice(p, c),
                        xs_block(p + c),
                        start=(c == 0),
                        stop=(c == NCH - 1),
                    )
                g = out_group_of[p]
                j = out_off_of[p]
                nc.vector.tensor_scalar_add(
                    out_tiles[g][:, j * 64:(j + 1) * 64], ps[:],
                    bs_t[:, p:p + 1])
                if j == OUT_SIZES[g] - 1:
                    eng = nc.scalar if g % 2 == 1 or g == OUT_GROUPS - 1 \
                        else nc.sync
                    b0 = int(out_base[g])
                    eng.dma_start(
                        out_d[:, b0 * 64:(b0 + OUT_SIZES[g]) * 64],
                        out_tiles[g][:])

    nc.compile()
    _cached_nc = nc
    return nc


def _prep_core_inputs(xpad, weight, bias, cr):
    l0 = LS * cr
    # xs[dj*64+i, t*64+b] = xpad[b, i, l0+2t+dj]
    xsl = xpad[:, :, l0:l0 + 2 * NT]                       # [b, i, 72]
    xs = np.ascontiguousarray(
        xsl.reshape(B, CI, NT, 2).transpose(3, 1, 2, 0)    # [dj, i, t, b]
    ).reshape(128, NT * 64)
    h = NT // 2
    xsa = np.ascontiguousarray(xs[:, :h * 64])
    xsb = np.ascontiguousarray(xs[:, h * 64:])

    # ws[dj*64+i, (p*NCH+c)*128 + l2*64 + o] = w[i,o,2c+dj-l2, l0+2p+l2]
    wsarr = np.zeros((NPAIR, 2, CI, NCH, 2, CO), np.float32)
    for c in range(NCH):
        for dj in range(2):
            for l2 in range(2):
                k = 2 * c + dj - l2
                if 0 <= k < KS:
                    wsl = weight[:, :, k, l0 + l2:l0 + l2 + 64:2]  # [i,o,p]
                    wsarr[:, dj, :, c, l2, :] = wsl.transpose(2, 0, 1)
    ws_rows = np.ascontiguousarray(
        wsarr.transpose(1, 2, 0, 3, 4, 5)        # [dj, i, p, c, l2, o]
    ).reshape(128, NPAIR * NCH * 128)
    # group-major contiguous blocks, each [128, gsz*NCH*128] row-major
    blocks = []
    c0 = 0
    for gsz in WSIZES:
        blocks.append(np.ascontiguousarray(
            ws_rows[:, c0 * NCH * 128:(c0 + gsz) * NCH * 128]).reshape(-1))
        c0 += gsz
    ws = np.concatenate(blocks)

    # bs[l2*64+o, p] = bias[o, l0+2p+l2]
    bs = np.ascontiguousarray(
        bias[:, l0:l0 + LS].reshape(CO, NPAIR, 2).transpose(2, 0, 1)
    ).reshape(128, NPAIR)

    return {
        "xsa": xsa.astype(bfloat16),
        "xsb": xsb.astype(bfloat16),
        "ws": ws.astype(bfloat16),
        "bs": bs.astype(np.float32),
    }


def kernel(x, weight, bias):
    global LAST_RESULT
    x = np.asarray(x, np.float32)
    weight = np.asarray(weight, np.float32)
    bias = np.asarray(bias, np.float32)

    nc = _build_nc()

    xpad = np.zeros((B, CI, S + 2 * PAD), np.float32)
    xpad[:, :, PAD:PAD + S] = x

    in_maps = [_prep_core_inputs(xpad, weight, bias, cr)
               for cr in range(NCORES)]

    kw = dict(TRACE_KW)
    if TRACE:
        kw.setdefault("trace", True)
    res = run_bass_kernel_spmd(nc, in_maps, list(range(NCORES)), **kw)
    LAST_RESULT = res

    out = np.empty((B, CO, L), np.float32)
    for cr in range(NCORES):
        r = np.asarray(res.results[cr]["out"]).astype(np.float32)  # [128, 2048]
        out[:, :, LS * cr:LS * (cr + 1)] = (
            r.reshape(2, CO, NPAIR, B).transpose(3, 1, 2, 0).reshape(B, CO, LS)
        )
    return out
